# revision 12
# baseline (speedup 1.0000x reference)
"""BernNet (nn_BernNet_9543417332146) Trainium2 kernel.

Reference computation:
    h = relu(x @ W1 + b1) @ W2 + b2                      (MLP head)
    out = sum_j  C(K,j)/2^K * relu(temp)_j * L^j (2I-L)^{K-j} h
  with L = I - A  (A = sym-normalized adjacency), evaluated by the
  reference via 65 sparse matvecs.

All terms are polynomials in A and commute, so
    out = p(A) h,   p(l) = sum_j c_j T_j (1-l)^j (1+l)^{K-j}
a degree-K polynomial whose coefficients depend only on `temp`.  For
temp = ones (the initialized BernNet parameters), the binomial sum
telescopes:  sum_j C(K,j) (1-l)^j (1+l)^{K-j} = 2^K  =>  p == 1, i.e.
the whole graph propagation is the identity and out == h exactly.

This kernel computes the polynomial coefficients from `temp` at runtime
with exact integer arithmetic, runs the MLP on all 8 NeuronCores
(nodes row-sharded, weights replicated), and only performs sparse
matvec work for the (never-initialized) case of nonzero higher-degree
coefficients, via a Horner evaluation needing deg(p) matvecs instead of
the reference's 65.
"""

import numpy as np
from math import comb

N_NODES = 50000
FEATURES = 64
NHID = 128
NCORES = 8
ROWS_PER_CORE = 6400          # 8 * 6400 = 51200 >= 50000 (zero padded)
HALF = ROWS_PER_CORE // 2     # 3200: rows split into two 64-partition halves
CHUNK = 512                   # matmul moving-operand free size (fp32 max)

_nc_cache = {}


def _bern_poly_coefs(temp):
    """Coefficients a_m of p(A) = sum_m a_m A^m for the BernNet filter.

    p(l) = sum_j [C(K,j)/2^K] * relu(temp_j) * (1-l)^j (1+l)^{K-j}.
    The inner binomial products are exact integers, so for temp = ones
    the higher coefficients cancel to exactly 0.0 in float arithmetic.
    """
    k = temp.shape[0] - 1
    T = np.maximum(np.asarray(temp, np.float64), 0.0)
    a = np.zeros(k + 1)
    for j in range(k + 1):
        tj = T[j]
        if tj == 0.0:
            continue
        for m in range(k + 1):
            s = 0
            for p in range(max(0, m - (k - j)), min(j, m) + 1):
                s += (-1) ** p * comb(j, p) * comb(k - j, m - p)
            a[m] += (comb(k, j) * s) * tj / float(2**k)
    return a


# Blob column layout (everything rides one input DMA so every consumer
# waits on a single DMA semaphore — the TRN2 fused Matmult+LDWEIGHTS
# instruction only supports ONE sync wait):
#   [0, HALF)            x shard, packed (see _pack_shard)
#   [HALF, HALF+128)     W1 duplicated on both partition halves
#   HALF+128             b1 (per-partition)
#   [HALF+129, HALF+193) W2
#   HALF+193             b2 duplicated on both partition halves
C_W1 = HALF
C_B1 = HALF + NHID
C_W2 = C_B1 + 1
C_B2 = C_W2 + FEATURES
BLOBW = C_B2 + 1


def _build_mlp_nc(repeat=1):
    """SPMD per-core program: y = (relu(x@W1+b1))@W2+b2 for a 6400-row
    shard, x packed transposed as [128, HALF] (partitions 0..63 = features
    of rows 0..HALF-1, partitions 64..127 = features of the other half) so
    the big in/out DMAs use all 128 partitions fully contiguously.

    Per-chunk epilogue ops alternate between the scalar and vector
    engines so that each matmul's RAW + psum-WAR dependencies land on one
    semaphore (PE matmuls can encode only a single sync wait).
    """
    import concourse.bass as bass
    import concourse.bacc as bacc
    import concourse.mybir as mybir
    from concourse.tile import TileContext

    f32 = mybir.dt.float32
    relu = mybir.ActivationFunctionType.Relu
    copyf = mybir.ActivationFunctionType.Identity
    add_op = mybir.AluOpType.add
    max_op = mybir.AluOpType.max
    # Bacc (not bare Bass): its lowering legalizes multi-wait instructions
    # into fused event-semaphore sequences the TRN2 encoders accept.
    nc = bacc.Bacc(None, target_bir_lowering=False)

    blob = nc.dram_tensor("blob", [128, BLOBW], f32, kind="ExternalInput")
    yt = nc.dram_tensor("yt", [128, HALF], f32, kind="ExternalOutput")

    with TileContext(nc) as tc:
        with (
            tc.tile_pool(name="io", bufs=1) as iopool,
            tc.tile_pool(name="work", bufs=2) as wpool,
            tc.tile_pool(name="psum", bufs=2, space=bass.MemorySpace.PSUM) as ppool,
        ):
            bt = iopool.tile([128, BLOBW], f32, tag="blob")
            y0 = iopool.tile([64, HALF], f32, tag="y0")
            y1 = iopool.tile([64, HALF], f32, tag="y1")
            b1t = bt[:, C_B1 : C_B1 + 1]
            w2t = bt[:, C_W2 : C_W2 + FEATURES]
            b2t = bt[:64, C_B2 : C_B2 + 1]

            # repeat>1 re-runs the whole body (DMAs included) inside one
            # NEFF — used by the test harness to measure steady-state HW
            # time via (T(R2)-T(R1))/(R2-R1), cancelling dispatch overhead.
            for _rep in range(repeat):
                nc.sync.dma_start(bt[:], blob[:])
                cnt = 0
                for half in range(2):
                    p0 = 64 * half
                    xs = bt[p0 : p0 + 64, 0:HALF]
                    w1s = bt[p0 : p0 + 64, C_W1 : C_W1 + NHID]
                    ys = y0 if half == 0 else y1
                    for c0 in range(0, HALF, CHUNK):
                        w = min(CHUNK, HALF - c0)
                        ps1 = ppool.tile([NHID, CHUNK], f32, tag="ps1")
                        nc.tensor.matmul(
                            ps1[:, :w], w1s, xs[:, c0 : c0 + w], start=True, stop=True
                        )
                        rt = wpool.tile([NHID, CHUNK], f32, tag="rt")
                        if cnt % 2 == 0:
                            nc.scalar.activation(rt[:, :w], ps1[:, :w], relu, bias=b1t)
                        else:
                            nc.vector.tensor_scalar(
                                rt[:, :w], ps1[:, :w], b1t, 0.0, add_op, max_op
                            )
                        ps2 = ppool.tile([FEATURES, CHUNK], f32, tag="ps2")
                        nc.tensor.matmul(
                            ps2[:, :w], w2t, rt[:, :w], start=True, stop=True
                        )
                        if cnt % 2 == 0:
                            nc.scalar.activation(
                                ys[:, c0 : c0 + w], ps2[:, :w], copyf, bias=b2t
                            )
                        else:
                            nc.vector.tensor_scalar_add(
                                ys[:, c0 : c0 + w], ps2[:, :w], b2t
                            )
                        cnt += 1

                nc.sync.dma_start(yt[:64, :], y0[:])
                nc.sync.dma_start(yt[64:, :], y1[:])
    nc.compile()
    return nc


def _pack_shard(x_pad, c):
    xs = x_pad[c * ROWS_PER_CORE : (c + 1) * ROWS_PER_CORE]   # (6400, 64)
    xtc = xs.T                                                # (64, 6400)
    return np.ascontiguousarray(
        np.concatenate([xtc[:, :HALF], xtc[:, HALF:]], axis=0)
    )  # (128, HALF)


def _unpack_shard(y):
    # (128, HALF) -> (6400, 64)
    return np.concatenate([y[:64, :], y[64:, :]], axis=1).T


def _mlp_numpy(x, W1, b1, W2, b2):
    return np.maximum(x @ W1 + b1, 0.0) @ W2 + b2


def _make_runner(nc, n_cores=NCORES):
    """Persistent jitted executor for a prebuilt Bass module (mirrors
    bass2jax.run_bass_via_pjrt's sharded path, but jit-compiled once and
    without donation so it can be invoked repeatedly for timing).

    Returns (fn, in_names, out_names, out_avals): fn takes the
    axis-0-concatenated per-core inputs followed by concatenated zero
    output buffers and returns concatenated outputs.
    """
    import jax
    import concourse.mybir as mybir
    from concourse import bass2jax
    from jax.experimental.shard_map import shard_map
    from jax.sharding import Mesh, PartitionSpec

    bass2jax.install_neuronx_cc_hook()
    partition_name = nc.partition_id_tensor.name if nc.partition_id_tensor else None
    in_names, out_names, out_avals = [], [], []
    for alloc in nc.m.functions[0].allocations:
        if not isinstance(alloc, mybir.MemoryLocationSet):
            continue
        name = alloc.memorylocations[0].name
        if alloc.kind == "ExternalInput":
            if name != partition_name:
                in_names.append(name)
        elif alloc.kind == "ExternalOutput":
            out_names.append(name)
            out_avals.append(
                jax.core.ShapedArray(
                    tuple(alloc.tensor_shape), mybir.dt.np(alloc.dtype)
                )
            )
    n_params = len(in_names)
    all_in = list(in_names) + list(out_names)
    if partition_name is not None:
        all_in.append(partition_name)

    def _body(*args):
        operands = list(args)
        if partition_name is not None:
            operands.append(bass2jax.partition_id_tensor())
        return tuple(
            bass2jax._bass_exec_p.bind(
                *operands,
                out_avals=tuple(out_avals),
                in_names=tuple(all_in),
                out_names=tuple(out_names),
                lowering_input_output_aliases=(),
                sim_require_finite=True,
                sim_require_nnan=True,
                nc=nc,
            )
        )

    import numpy as _np

    devices = jax.devices()[:n_cores]
    mesh = Mesh(_np.asarray(devices), ("core",))
    nin = n_params + len(out_names)
    fn = jax.jit(
        shard_map(
            _body,
            mesh=mesh,
            in_specs=(PartitionSpec("core"),) * nin,
            out_specs=(PartitionSpec("core"),) * len(out_names),
            check_rep=False,
        ),
        keep_unused=True,
    )
    return fn, in_names, out_names, out_avals


def _mlp_trn(x, W1, b1, W2, b2, trace=False):
    """Run the MLP row-sharded across the 8 NeuronCores. Returns
    (h, exec_time_ns) — exec_time_ns is only populated when an NTFF
    profiling hook is available (trace=True); the test harness instead
    measures HW time via inner-repeat deltas."""
    from concourse.bass_utils import run_bass_kernel_spmd

    n = x.shape[0]
    if "nc" not in _nc_cache:
        _nc_cache["nc"] = _build_mlp_nc()
    nc = _nc_cache["nc"]

    n_pad = NCORES * ROWS_PER_CORE
    x_pad = np.zeros((n_pad, FEATURES), np.float32)
    x_pad[:n] = x

    consts = np.zeros((128, BLOBW - HALF), np.float32)
    consts[:, C_W1 - HALF : C_W1 - HALF + NHID] = np.concatenate([W1, W1], axis=0)
    consts[:, C_B1 - HALF] = b1
    consts[:, C_W2 - HALF : C_W2 - HALF + FEATURES] = W2
    consts[:, C_B2 - HALF] = np.concatenate([b2, b2])

    def blob_for(c):
        blob = np.empty((128, BLOBW), np.float32)
        blob[:, :HALF] = _pack_shard(x_pad, c)
        blob[:, HALF:] = consts
        return blob

    in_maps = [{"blob": blob_for(c)} for c in range(NCORES)]
    res = run_bass_kernel_spmd(nc, in_maps, list(range(NCORES)), trace=trace)
    h = np.empty((n_pad, FEATURES), np.float32)
    for c in range(NCORES):
        h[c * ROWS_PER_CORE : (c + 1) * ROWS_PER_CORE] = _unpack_shard(
            res.results[c]["yt"]
        )
    return h[:n], res.exec_time_ns


def kernel(x, edge_index, W1, b1, W2, b2, temp):
    x = np.asarray(x, np.float32)
    W1 = np.asarray(W1, np.float32)
    b1 = np.asarray(b1, np.float32)
    W2 = np.asarray(W2, np.float32)
    b2 = np.asarray(b2, np.float32)
    temp = np.asarray(temp, np.float32)
    n = x.shape[0]

    a = _bern_poly_coefs(temp)

    if x.shape == (N_NODES, FEATURES) and W1.shape == (FEATURES, NHID):
        try:
            h, _ = _mlp_trn(x, W1, b1, W2, b2)
        except Exception as e:  # infrastructure failure only — stay correct
            print(f"WARNING: TRN MLP path failed ({type(e).__name__}: {e}); "
                  f"falling back to numpy")
            h = _mlp_numpy(x, W1, b1, W2, b2)
    else:
        h = _mlp_numpy(x, W1, b1, W2, b2)

    deg = 0
    for m in range(len(a) - 1, 0, -1):
        if a[m] != 0.0:
            deg = m
            break

    if deg == 0:
        out = h if a[0] == 1.0 else a[0] * h
        return np.ascontiguousarray(out.astype(np.float32))

    # General path (temp != initialized ones): Horner with deg(p) sparse
    # matvecs. Unreachable for the shipped problem instance.
    src = np.asarray(edge_index[0], np.int64)
    dst = np.asarray(edge_index[1], np.int64)
    deg_out = np.bincount(src, minlength=n).astype(np.float32)
    dinv = np.where(deg_out > 0, 1.0 / np.sqrt(np.maximum(deg_out, 1.0)), 0.0).astype(
        np.float32
    )
    w_edge = (dinv[src] * dinv[dst]).astype(np.float32)

    from scipy.sparse import coo_matrix

    A = coo_matrix((w_edge, (dst, src)), shape=(n, n)).tocsr()
    z = (a[deg] * h).astype(np.float32)
    for m in range(deg - 1, -1, -1):
        z = (A @ z + a[m] * h).astype(np.float32)
    return np.ascontiguousarray(z.astype(np.float32))


# revision 34
# speedup vs baseline: 5.2934x; 5.2934x over previous
"""BernNet (nn_BernNet_9543417332146) Trainium2 kernel.

Reference computation:
    h = relu(x @ W1 + b1) @ W2 + b2                      (MLP head)
    out = sum_j  C(K,j)/2^K * relu(temp)_j * L^j (2I-L)^{K-j} h
  with L = I - A  (A = sym-normalized adjacency), evaluated by the
  reference via 65 sparse matvecs.

All terms are polynomials in A and commute, so
    out = p(A) h,   p(l) = sum_j c_j T_j (1-l)^j (1+l)^{K-j}
a degree-K polynomial whose coefficients depend only on `temp`.  For
temp = ones (the initialized BernNet parameters), the binomial sum
telescopes:  sum_j C(K,j) (1-l)^j (1+l)^{K-j} = 2^K  =>  p == 1, i.e.
the whole graph propagation is the identity and out == h exactly.

This kernel computes the polynomial coefficients from `temp` at runtime
with exact integer arithmetic, runs the MLP on all 8 NeuronCores
(nodes row-sharded, weights replicated), and only performs sparse
matvec work for the (never-initialized) case of nonzero higher-degree
coefficients, via a Horner evaluation needing deg(p) matvecs instead of
the reference's 65.
"""

import numpy as np
from math import comb

N_NODES = 50000
FEATURES = 64
NHID = 128
NCORES = 8
ROWS_PER_CORE = 6400          # 8 * 6400 = 51200 >= 50000 (zero padded)
HALF = ROWS_PER_CORE // 2     # 3200: rows split into two 64-partition halves
CHUNK = 512                   # matmul moving-operand free size (fp32 max)

_nc_cache = {}


def _bern_poly_coefs(temp):
    """Coefficients a_m of p(A) = sum_m a_m A^m for the BernNet filter.

    p(l) = sum_j [C(K,j)/2^K] * relu(temp_j) * (1-l)^j (1+l)^{K-j}.
    The inner binomial products are exact integers, so for temp = ones
    the higher coefficients cancel to exactly 0.0 in float arithmetic.
    """
    k = temp.shape[0] - 1
    T = np.maximum(np.asarray(temp, np.float64), 0.0)
    a = np.zeros(k + 1)
    for j in range(k + 1):
        tj = T[j]
        if tj == 0.0:
            continue
        for m in range(k + 1):
            s = 0
            for p in range(max(0, m - (k - j)), min(j, m) + 1):
                s += (-1) ** p * comb(j, p) * comb(k - j, m - p)
            a[m] += (comb(k, j) * s) * tj / float(2**k)
    return a


# Blob column layout. Constants come FIRST so the first (small) input
# DMA covers them, then the x shard streams in CHUNK-sized pieces that
# unblock compute as they land:
#   [0, 128)        W1 duplicated on both partition halves
#   128             b1 (per-partition)
#   [129, 193)      W2
#   193             b2 duplicated on both partition halves
#   [194, 194+HALF) x shard, packed (see _pack_shard)
C_W1 = 0
C_B1 = NHID
C_W2 = C_B1 + 1
C_B2 = C_W2 + FEATURES
C_X = C_B2 + 1
BLOBW = C_X + HALF


def _build_mlp_nc(repeat=1):
    """SPMD per-core program: y = (relu(x@W1+b1))@W2+b2 for a 6400-row
    shard, x packed transposed as [128, HALF] (partitions 0..63 = features
    of rows 0..HALF-1, partitions 64..127 = features of the other half) so
    the big in/out DMAs use all 128 partitions fully contiguously.

    Per-chunk epilogue ops alternate between the scalar and vector
    engines so that each matmul's RAW + psum-WAR dependencies land on one
    semaphore (PE matmuls can encode only a single sync wait).
    """
    import concourse.bass as bass
    import concourse.bacc as bacc
    import concourse.mybir as mybir
    from concourse.tile import TileContext

    f32 = mybir.dt.float32
    f32r = mybir.dt.float32r
    relu = mybir.ActivationFunctionType.Relu
    copyf = mybir.ActivationFunctionType.Identity
    add_op = mybir.AluOpType.add
    max_op = mybir.AluOpType.max
    # Bacc (not bare Bass): its lowering legalizes multi-wait instructions
    # into fused event-semaphore sequences the TRN2 encoders accept.
    nc = bacc.Bacc(None, target_bir_lowering=False)

    # Blob is float32r end-to-end (host pre-rounds to the 12-mantissa-bit
    # FP32R grid) so the fast-path matmuls see "rounded" producers.
    blob = nc.dram_tensor("blob", [128, BLOBW], f32r, kind="ExternalInput")
    yt = nc.dram_tensor("yt", [128, HALF], f32, kind="ExternalOutput")

    with TileContext(nc) as tc:
        with (
            tc.tile_pool(name="io", bufs=1) as iopool,
            tc.tile_pool(name="work", bufs=4) as wpool,
            tc.tile_pool(name="yout", bufs=7) as ypool,
            tc.tile_pool(name="psum", bufs=4, space=bass.MemorySpace.PSUM) as ppool,
            tc.tile_pool(name="psum2", bufs=3, space=bass.MemorySpace.PSUM) as ppool2,
        ):
            bt = iopool.tile([128, BLOBW], f32r, tag="blob")
            b1t = bt[:, C_B1 : C_B1 + 1].bitcast(f32)
            w2t = bt[:, C_W2 : C_W2 + FEATURES]

            b2t = bt[:64, C_B2 : C_B2 + 1].bitcast(f32)

            # repeat>1 re-runs the whole body (DMAs included) inside one
            # NEFF — used by the test harness to measure steady-state HW
            # time via (T(R2)-T(R1))/(R2-R1), cancelling dispatch overhead.
            for _rep in range(repeat):
                # first piece = consts + first x chunk (small, lands fast,
                # unblocks chunk-0 compute); rest streams in ~1K pieces
                for p0c, p1c in (
                    (0, C_X + CHUNK),
                    (C_X + CHUNK, C_X + CHUNK + 1024),
                    (C_X + CHUNK + 1024, C_X + CHUNK + 2048),
                    (C_X + CHUNK + 2048, BLOBW),
                ):
                    nc.sync.dma_start(bt[:, p0c:p1c], blob[:, p0c:p1c])

                cnt = 0
                for ci, c0 in enumerate(range(0, HALF, CHUNK)):
                    w = min(CHUNK, HALF - c0)
                    rts = []
                    for half in range(2):
                        p0 = 64 * half
                        # float32r: fp32 rounded to 12 mantissa bits;
                        # streams through the PE at 1 cycle/row (vs 4 for
                        # plain fp32) when N >= 256.
                        xs = bt[p0 : p0 + 64, C_X + c0 : C_X + c0 + w]
                        w1s = bt[p0 : p0 + 64, C_W1 : C_W1 + NHID]
                        ps1 = ppool.tile([NHID, CHUNK], f32, tag="ps1")
                        nc.tensor.matmul(
                            ps1[:, :w], w1s, xs, start=True, stop=True
                        )
                        rt = wpool.tile([NHID, CHUNK], f32r, tag="rt")
                        if cnt % 2 == 0:
                            nc.scalar.activation(rt[:, :w], ps1[:, :w], relu, bias=b1t)
                        else:
                            nc.vector.tensor_scalar(
                                rt[:, :w], ps1[:, :w], b1t, 0.0, add_op, max_op
                            )
                        rts.append(rt)
                        cnt += 1
                    # per-half [64, w] PSUM tiles (matmul output must start
                    # at partition 0), but both epilogues land in ONE
                    # [128, w] SBUF tile so the store runs once per chunk
                    # at full partition width
                    yc = ypool.tile([128, CHUNK], f32, tag="yc")
                    for half in range(2):
                        ps2 = ppool2.tile([FEATURES, CHUNK], f32, tag="ps2")
                        nc.tensor.matmul(
                            ps2[:, :w], w2t, rts[half][:, :w], start=True, stop=True
                        )
                        ycs = yc[64 * half : 64 * half + FEATURES, :w]
                        if (ci + half) % 2 == 0:
                            nc.scalar.activation(ycs, ps2[:, :w], copyf, bias=b2t)
                        else:
                            nc.vector.tensor_scalar_add(ycs, ps2[:, :w], b2t)
                    # SP's queue is idle once the 4 input DMAs are issued
                    nc.sync.dma_start(yt[:, c0 : c0 + w], yc[:, :w])
    nc.compile()
    return nc


def _round_fp32r(a):
    """Round float32 array to the FP32R grid (12 mantissa bits, RNE) —
    matches the compiler's fp32_to_fp32r."""
    bits = np.ascontiguousarray(a, np.float32).view(np.uint32).copy()
    bits += 0x7FF + ((bits >> 12) & 1)
    bits &= np.uint32(0xFFFFF000)
    return bits.view(np.float32)


def _pack_shard(x_pad, c):
    xs = x_pad[c * ROWS_PER_CORE : (c + 1) * ROWS_PER_CORE]   # (6400, 64)
    xtc = xs.T                                                # (64, 6400)
    return np.ascontiguousarray(
        np.concatenate([xtc[:, :HALF], xtc[:, HALF:]], axis=0)
    )  # (128, HALF)


def _unpack_shard(y):
    # (128, HALF) -> (6400, 64)
    return np.concatenate([y[:64, :], y[64:, :]], axis=1).T


def _mlp_numpy(x, W1, b1, W2, b2):
    return np.maximum(x @ W1 + b1, 0.0) @ W2 + b2


def _make_runner(nc, n_cores=NCORES):
    """Persistent jitted executor for a prebuilt Bass module (mirrors
    bass2jax.run_bass_via_pjrt's sharded path, but jit-compiled once and
    without donation so it can be invoked repeatedly for timing).

    Returns (fn, in_names, out_names, out_avals): fn takes the
    axis-0-concatenated per-core inputs followed by concatenated zero
    output buffers and returns concatenated outputs.
    """
    import jax
    import concourse.mybir as mybir
    from concourse import bass2jax
    from jax.experimental.shard_map import shard_map
    from jax.sharding import Mesh, PartitionSpec

    bass2jax.install_neuronx_cc_hook()
    partition_name = nc.partition_id_tensor.name if nc.partition_id_tensor else None
    in_names, out_names, out_avals = [], [], []
    for alloc in nc.m.functions[0].allocations:
        if not isinstance(alloc, mybir.MemoryLocationSet):
            continue
        name = alloc.memorylocations[0].name
        if alloc.kind == "ExternalInput":
            if name != partition_name:
                in_names.append(name)
        elif alloc.kind == "ExternalOutput":
            out_names.append(name)
            out_avals.append(
                jax.core.ShapedArray(
                    tuple(alloc.tensor_shape), mybir.dt.np(alloc.dtype)
                )
            )
    n_params = len(in_names)
    all_in = list(in_names) + list(out_names)
    if partition_name is not None:
        all_in.append(partition_name)

    def _body(*args):
        operands = list(args)
        if partition_name is not None:
            operands.append(bass2jax.partition_id_tensor())
        return tuple(
            bass2jax._bass_exec_p.bind(
                *operands,
                out_avals=tuple(out_avals),
                in_names=tuple(all_in),
                out_names=tuple(out_names),
                lowering_input_output_aliases=(),
                sim_require_finite=True,
                sim_require_nnan=True,
                nc=nc,
            )
        )

    import numpy as _np

    devices = jax.devices()[:n_cores]
    mesh = Mesh(_np.asarray(devices), ("core",))
    nin = n_params + len(out_names)
    fn = jax.jit(
        shard_map(
            _body,
            mesh=mesh,
            in_specs=(PartitionSpec("core"),) * nin,
            out_specs=(PartitionSpec("core"),) * len(out_names),
            check_rep=False,
        ),
        keep_unused=True,
    )
    return fn, in_names, out_names, out_avals


def _mlp_trn(x, W1, b1, W2, b2, trace=False):
    """Run the MLP row-sharded across the 8 NeuronCores. Returns
    (h, exec_time_ns) — exec_time_ns is only populated when an NTFF
    profiling hook is available (trace=True); the test harness instead
    measures HW time via inner-repeat deltas."""
    from concourse.bass_utils import run_bass_kernel_spmd

    n = x.shape[0]
    if "nc" not in _nc_cache:
        _nc_cache["nc"] = _build_mlp_nc()
    nc = _nc_cache["nc"]

    n_pad = NCORES * ROWS_PER_CORE
    x_pad = np.zeros((n_pad, FEATURES), np.float32)
    x_pad[:n] = x

    consts = np.zeros((128, C_X), np.float32)
    consts[:, C_W1 : C_W1 + NHID] = np.concatenate([W1, W1], axis=0)
    consts[:, C_B1] = b1
    consts[:, C_W2 : C_W2 + FEATURES] = W2
    consts[:, C_B2] = np.concatenate([b2, b2])

    def blob_for(c):
        blob = np.empty((128, BLOBW), np.float32)
        blob[:, :C_X] = consts
        blob[:, C_X:] = _pack_shard(x_pad, c)
        return _round_fp32r(blob)

    in_maps = [{"blob": blob_for(c)} for c in range(NCORES)]
    res = run_bass_kernel_spmd(nc, in_maps, list(range(NCORES)), trace=trace)
    h = np.empty((n_pad, FEATURES), np.float32)
    for c in range(NCORES):
        h[c * ROWS_PER_CORE : (c + 1) * ROWS_PER_CORE] = _unpack_shard(
            res.results[c]["yt"]
        )
    return h[:n], res.exec_time_ns


def kernel(x, edge_index, W1, b1, W2, b2, temp):
    x = np.asarray(x, np.float32)
    W1 = np.asarray(W1, np.float32)
    b1 = np.asarray(b1, np.float32)
    W2 = np.asarray(W2, np.float32)
    b2 = np.asarray(b2, np.float32)
    temp = np.asarray(temp, np.float32)
    n = x.shape[0]

    a = _bern_poly_coefs(temp)

    if x.shape == (N_NODES, FEATURES) and W1.shape == (FEATURES, NHID):
        try:
            h, _ = _mlp_trn(x, W1, b1, W2, b2)
        except Exception as e:  # infrastructure failure only — stay correct
            print(f"WARNING: TRN MLP path failed ({type(e).__name__}: {e}); "
                  f"falling back to numpy")
            h = _mlp_numpy(x, W1, b1, W2, b2)
    else:
        h = _mlp_numpy(x, W1, b1, W2, b2)

    deg = 0
    for m in range(len(a) - 1, 0, -1):
        if a[m] != 0.0:
            deg = m
            break

    if deg == 0:
        out = h if a[0] == 1.0 else a[0] * h
        return np.ascontiguousarray(out.astype(np.float32))

    # General path (temp != initialized ones): Horner with deg(p) sparse
    # matvecs. Unreachable for the shipped problem instance.
    src = np.asarray(edge_index[0], np.int64)
    dst = np.asarray(edge_index[1], np.int64)
    deg_out = np.bincount(src, minlength=n).astype(np.float32)
    dinv = np.where(deg_out > 0, 1.0 / np.sqrt(np.maximum(deg_out, 1.0)), 0.0).astype(
        np.float32
    )
    w_edge = (dinv[src] * dinv[dst]).astype(np.float32)

    from scipy.sparse import coo_matrix

    A = coo_matrix((w_edge, (dst, src)), shape=(n, n)).tocsr()
    z = (a[deg] * h).astype(np.float32)
    for m in range(deg - 1, -1, -1):
        z = (A @ z + a[m] * h).astype(np.float32)
    return np.ascontiguousarray(z.astype(np.float32))


# revision 38
# speedup vs baseline: 15.7211x; 2.9699x over previous
"""BernNet (nn_BernNet_9543417332146) Trainium2 kernel.

Reference computation:
    h = relu(x @ W1 + b1) @ W2 + b2                      (MLP head)
    out = sum_j  C(K,j)/2^K * relu(temp)_j * L^j (2I-L)^{K-j} h
  with L = I - A  (A = sym-normalized adjacency), evaluated by the
  reference via 65 sparse matvecs.

All terms are polynomials in A and commute, so
    out = p(A) h,   p(l) = sum_j c_j T_j (1-l)^j (1+l)^{K-j}
a degree-K polynomial whose coefficients depend only on `temp`.  For
temp = ones (the initialized BernNet parameters), the binomial sum
telescopes:  sum_j C(K,j) (1-l)^j (1+l)^{K-j} = 2^K  =>  p == 1, i.e.
the whole graph propagation is the identity and out == h exactly.

This kernel computes the polynomial coefficients from `temp` at runtime
with exact integer arithmetic, runs the MLP on all 8 NeuronCores
(nodes row-sharded, weights replicated), and only performs sparse
matvec work for the (never-initialized) case of nonzero higher-degree
coefficients, via a Horner evaluation needing deg(p) matvecs instead of
the reference's 65.
"""

import numpy as np
from math import comb

N_NODES = 50000
FEATURES = 64
NHID = 128
NCORES = 8
ROWS_PER_CORE = 6400          # 8 * 6400 = 51200 >= 50000 (zero padded)
HALF = ROWS_PER_CORE // 2     # 3200: rows split into two 64-partition halves
CHUNK = 512                   # matmul moving-operand free size (fp32 max)

_nc_cache = {}


def _bern_poly_coefs(temp):
    """Coefficients a_m of p(A) = sum_m a_m A^m for the BernNet filter.

    p(l) = sum_j [C(K,j)/2^K] * relu(temp_j) * (1-l)^j (1+l)^{K-j}.
    The inner binomial products are exact integers, so for temp = ones
    the higher coefficients cancel to exactly 0.0 in float arithmetic.
    """
    k = temp.shape[0] - 1
    T = np.maximum(np.asarray(temp, np.float64), 0.0)
    a = np.zeros(k + 1)
    for j in range(k + 1):
        tj = T[j]
        if tj == 0.0:
            continue
        for m in range(k + 1):
            s = 0
            for p in range(max(0, m - (k - j)), min(j, m) + 1):
                s += (-1) ** p * comb(j, p) * comb(k - j, m - p)
            a[m] += (comb(k, j) * s) * tj / float(2**k)
    return a


# Blob column layout. Constants come FIRST so the first (small) input
# DMA covers them, then the x shard streams in CHUNK-sized pieces that
# unblock compute as they land:
#   [0, 128)        W1 duplicated on both partition halves
#   128             b1 (per-partition)
#   [129, 193)      W2
#   193             b2 duplicated on both partition halves
#   [194, 194+HALF) x shard, packed (see _pack_shard)
C_W1 = 0
C_B1 = NHID
C_W2 = C_B1 + 1
C_B2 = C_W2 + FEATURES
C_X = C_B2 + 1
BLOBW = C_X + HALF


def _build_mlp_nc(repeat=1):
    """SPMD per-core program: y = (relu(x@W1+b1))@W2+b2 for a 6400-row
    shard, x packed transposed as [128, HALF] (partitions 0..63 = features
    of rows 0..HALF-1, partitions 64..127 = features of the other half) so
    the big in/out DMAs use all 128 partitions fully contiguously.

    Relu and bias epilogues alternate between the scalar and vector
    engines to split the elementwise work across both.  Built on Bacc:
    its lowering legalizes multi-wait instructions (TRN2 compute
    instructions encode only a single sync wait) into event-semaphore
    sequences.
    """
    import concourse.bass as bass
    import concourse.bacc as bacc
    import concourse.mybir as mybir
    from concourse.tile import TileContext

    f32 = mybir.dt.float32
    f32r = mybir.dt.float32r
    relu = mybir.ActivationFunctionType.Relu
    copyf = mybir.ActivationFunctionType.Identity
    add_op = mybir.AluOpType.add
    max_op = mybir.AluOpType.max
    # Bacc (not bare Bass): its lowering legalizes multi-wait instructions
    # into fused event-semaphore sequences the TRN2 encoders accept.
    nc = bacc.Bacc(None, target_bir_lowering=False)

    # Blob is float32r end-to-end (host pre-rounds to the 12-mantissa-bit
    # FP32R grid) so the fast-path matmuls see "rounded" producers.
    blob = nc.dram_tensor("blob", [128, BLOBW], f32r, kind="ExternalInput")
    yt = nc.dram_tensor("yt", [128, HALF], f32, kind="ExternalOutput")

    with TileContext(nc) as tc:
        with (
            tc.tile_pool(name="io", bufs=1) as iopool,
            tc.tile_pool(name="work", bufs=4) as wpool,
            tc.tile_pool(name="yout", bufs=7) as ypool,
            tc.tile_pool(name="psum", bufs=4, space=bass.MemorySpace.PSUM) as ppool,
            tc.tile_pool(name="psum2", bufs=3, space=bass.MemorySpace.PSUM) as ppool2,
        ):
            bt = iopool.tile([128, BLOBW], f32r, tag="blob")
            b1t = bt[:, C_B1 : C_B1 + 1].bitcast(f32)
            w2t = bt[:, C_W2 : C_W2 + FEATURES]

            b2t = bt[:64, C_B2 : C_B2 + 1].bitcast(f32)

            # Pre-warm the ACT function-table (LoadActFuncSet ~1.3us)
            # before any data arrives, off the critical path.
            warm = wpool.tile([1, 1], f32, tag="warm")
            nc.vector.memset(warm[:], 0.0)
            nc.scalar.activation(warm[:], warm[:], relu)

            # chunk widths: small first chunk so compute starts as soon as
            # the small first DMA piece lands; all widths >= 256 keep the
            # fp32r matmuls on the 1-cycle/row path
            chunks = []
            c0 = 0
            for w in (256, 512, 512, 512, 512, 512, 384):
                chunks.append((c0, w))
                c0 += w
            assert c0 == HALF

            # repeat>1 re-runs the whole body (DMAs included) inside one
            # NEFF — used by the test harness to measure steady-state HW
            # time via (T(R2)-T(R1))/(R2-R1), cancelling dispatch overhead.
            for _rep in range(repeat):
                # first piece = consts + first x chunk; the rest streams in
                # ~1K-column pieces, split across the SP and ACT HWDGE
                # queues so issue latency overlaps
                pieces = (
                    (0, C_X + 256),
                    (C_X + 256, C_X + 1280),
                    (C_X + 1280, C_X + 2304),
                    (C_X + 2304, BLOBW),
                )
                for pi, (p0c, p1c) in enumerate(pieces):
                    eng = nc.sync if pi % 2 == 0 else nc.scalar
                    eng.dma_start(bt[:, p0c:p1c], blob[:, p0c:p1c])

                cnt = 0
                for ci, (c0, w) in enumerate(chunks):
                    rts = []
                    for half in range(2):
                        p0 = 64 * half
                        # float32r: fp32 rounded to 12 mantissa bits;
                        # streams through the PE at 1 cycle/row (vs 4 for
                        # plain fp32) when N >= 256.
                        xs = bt[p0 : p0 + 64, C_X + c0 : C_X + c0 + w]
                        w1s = bt[p0 : p0 + 64, C_W1 : C_W1 + NHID]
                        ps1 = ppool.tile([NHID, CHUNK], f32, tag="ps1")
                        nc.tensor.matmul(
                            ps1[:, :w], w1s, xs, start=True, stop=True
                        )
                        rt = wpool.tile([NHID, CHUNK], f32r, tag="rt")
                        if cnt % 2 == 0:
                            nc.scalar.activation(rt[:, :w], ps1[:, :w], relu, bias=b1t)
                        else:
                            nc.vector.tensor_scalar(
                                rt[:, :w], ps1[:, :w], b1t, 0.0, add_op, max_op
                            )
                        rts.append(rt)
                        cnt += 1
                    # per-half [64, w] PSUM tiles (matmul output must start
                    # at partition 0), but both epilogues land in ONE
                    # [128, w] SBUF tile so the store runs once per chunk
                    # at full partition width
                    yc = ypool.tile([128, CHUNK], f32, tag="yc")
                    for half in range(2):
                        ps2 = ppool2.tile([FEATURES, CHUNK], f32, tag="ps2")
                        nc.tensor.matmul(
                            ps2[:, :w], w2t, rts[half][:, :w], start=True, stop=True
                        )
                        ycs = yc[64 * half : 64 * half + FEATURES, :w]
                        if (ci + half) % 2 == 0:
                            nc.scalar.activation(ycs, ps2[:, :w], copyf, bias=b2t)
                        else:
                            nc.vector.tensor_scalar_add(ycs, ps2[:, :w], b2t)
                    # SP's queue is idle once the 4 input DMAs are issued
                    nc.sync.dma_start(yt[:, c0 : c0 + w], yc[:, :w])
    nc.compile()
    return nc


def _round_fp32r(a):
    """Round float32 array to the FP32R grid (12 mantissa bits, RNE) —
    matches the compiler's fp32_to_fp32r."""
    bits = np.ascontiguousarray(a, np.float32).view(np.uint32).copy()
    bits += 0x7FF + ((bits >> 12) & 1)
    bits &= np.uint32(0xFFFFF000)
    return bits.view(np.float32)


def _pack_shard(x_pad, c):
    xs = x_pad[c * ROWS_PER_CORE : (c + 1) * ROWS_PER_CORE]   # (6400, 64)
    xtc = xs.T                                                # (64, 6400)
    return np.ascontiguousarray(
        np.concatenate([xtc[:, :HALF], xtc[:, HALF:]], axis=0)
    )  # (128, HALF)


def _unpack_shard(y):
    # (128, HALF) -> (6400, 64)
    return np.concatenate([y[:64, :], y[64:, :]], axis=1).T


def _mlp_numpy(x, W1, b1, W2, b2):
    return np.maximum(x @ W1 + b1, 0.0) @ W2 + b2


def _make_runner(nc, n_cores=NCORES):
    """Persistent jitted executor for a prebuilt Bass module (mirrors
    bass2jax.run_bass_via_pjrt's sharded path, but jit-compiled once and
    without donation so it can be invoked repeatedly for timing).

    Returns (fn, in_names, out_names, out_avals): fn takes the
    axis-0-concatenated per-core inputs followed by concatenated zero
    output buffers and returns concatenated outputs.
    """
    import jax
    import concourse.mybir as mybir
    from concourse import bass2jax
    from jax.experimental.shard_map import shard_map
    from jax.sharding import Mesh, PartitionSpec

    bass2jax.install_neuronx_cc_hook()
    partition_name = nc.partition_id_tensor.name if nc.partition_id_tensor else None
    in_names, out_names, out_avals = [], [], []
    for alloc in nc.m.functions[0].allocations:
        if not isinstance(alloc, mybir.MemoryLocationSet):
            continue
        name = alloc.memorylocations[0].name
        if alloc.kind == "ExternalInput":
            if name != partition_name:
                in_names.append(name)
        elif alloc.kind == "ExternalOutput":
            out_names.append(name)
            out_avals.append(
                jax.core.ShapedArray(
                    tuple(alloc.tensor_shape), mybir.dt.np(alloc.dtype)
                )
            )
    n_params = len(in_names)
    all_in = list(in_names) + list(out_names)
    if partition_name is not None:
        all_in.append(partition_name)

    def _body(*args):
        operands = list(args)
        if partition_name is not None:
            operands.append(bass2jax.partition_id_tensor())
        return tuple(
            bass2jax._bass_exec_p.bind(
                *operands,
                out_avals=tuple(out_avals),
                in_names=tuple(all_in),
                out_names=tuple(out_names),
                lowering_input_output_aliases=(),
                sim_require_finite=True,
                sim_require_nnan=True,
                nc=nc,
            )
        )

    import numpy as _np

    devices = jax.devices()[:n_cores]
    mesh = Mesh(_np.asarray(devices), ("core",))
    nin = n_params + len(out_names)
    fn = jax.jit(
        shard_map(
            _body,
            mesh=mesh,
            in_specs=(PartitionSpec("core"),) * nin,
            out_specs=(PartitionSpec("core"),) * len(out_names),
            check_rep=False,
        ),
        keep_unused=True,
    )
    return fn, in_names, out_names, out_avals


def _mlp_trn(x, W1, b1, W2, b2, trace=False):
    """Run the MLP row-sharded across the 8 NeuronCores. Returns
    (h, exec_time_ns) — exec_time_ns is only populated when an NTFF
    profiling hook is available (trace=True); the test harness instead
    measures HW time via inner-repeat deltas."""
    from concourse.bass_utils import run_bass_kernel_spmd

    n = x.shape[0]
    if "nc" not in _nc_cache:
        _nc_cache["nc"] = _build_mlp_nc()
    nc = _nc_cache["nc"]

    n_pad = NCORES * ROWS_PER_CORE
    x_pad = np.zeros((n_pad, FEATURES), np.float32)
    x_pad[:n] = x

    consts = np.zeros((128, C_X), np.float32)
    consts[:, C_W1 : C_W1 + NHID] = np.concatenate([W1, W1], axis=0)
    consts[:, C_B1] = b1
    consts[:, C_W2 : C_W2 + FEATURES] = W2
    consts[:, C_B2] = np.concatenate([b2, b2])

    def blob_for(c):
        blob = np.empty((128, BLOBW), np.float32)
        blob[:, :C_X] = consts
        blob[:, C_X:] = _pack_shard(x_pad, c)
        return _round_fp32r(blob)

    in_maps = [{"blob": blob_for(c)} for c in range(NCORES)]
    res = run_bass_kernel_spmd(nc, in_maps, list(range(NCORES)), trace=trace)
    h = np.empty((n_pad, FEATURES), np.float32)
    for c in range(NCORES):
        h[c * ROWS_PER_CORE : (c + 1) * ROWS_PER_CORE] = _unpack_shard(
            res.results[c]["yt"]
        )
    return h[:n], res.exec_time_ns


def kernel(x, edge_index, W1, b1, W2, b2, temp):
    x = np.asarray(x, np.float32)
    W1 = np.asarray(W1, np.float32)
    b1 = np.asarray(b1, np.float32)
    W2 = np.asarray(W2, np.float32)
    b2 = np.asarray(b2, np.float32)
    temp = np.asarray(temp, np.float32)
    n = x.shape[0]

    a = _bern_poly_coefs(temp)

    if x.shape == (N_NODES, FEATURES) and W1.shape == (FEATURES, NHID):
        h = None
        for attempt in range(2):
            try:
                h, _ = _mlp_trn(x, W1, b1, W2, b2)
                break
            except Exception as e:  # infrastructure failure only
                print(f"WARNING: TRN MLP attempt {attempt} failed "
                      f"({type(e).__name__}: {e})")
        if h is None:  # stay correct even if the device is wedged
            print("WARNING: falling back to numpy MLP")
            h = _mlp_numpy(x, W1, b1, W2, b2)
    else:
        h = _mlp_numpy(x, W1, b1, W2, b2)

    deg = 0
    for m in range(len(a) - 1, 0, -1):
        if a[m] != 0.0:
            deg = m
            break

    if deg == 0:
        out = h if a[0] == 1.0 else a[0] * h
        return np.ascontiguousarray(out.astype(np.float32))

    # General path (temp != initialized ones): Horner with deg(p) sparse
    # matvecs. Unreachable for the shipped problem instance.
    src = np.asarray(edge_index[0], np.int64)
    dst = np.asarray(edge_index[1], np.int64)
    deg_out = np.bincount(src, minlength=n).astype(np.float32)
    dinv = np.where(deg_out > 0, 1.0 / np.sqrt(np.maximum(deg_out, 1.0)), 0.0).astype(
        np.float32
    )
    w_edge = (dinv[src] * dinv[dst]).astype(np.float32)

    try:
        from scipy.sparse import coo_matrix

        A = coo_matrix((w_edge, (dst, src)), shape=(n, n)).tocsr()
        anorm = lambda z: (A @ z).astype(np.float32)
    except ImportError:
        def anorm(z):
            out = np.zeros_like(z)
            np.add.at(out, dst, w_edge[:, None] * z[src])
            return out

    z = (a[deg] * h).astype(np.float32)
    for m in range(deg - 1, -1, -1):
        z = (anorm(z) + a[m] * h).astype(np.float32)
    return np.ascontiguousarray(z.astype(np.float32))
